# revision 1
# baseline (speedup 1.0000x reference)
"""Multi-head attention (B=2, S=4096, D=768, H=12) on 8 Trainium2 cores.

Sharding: (batch, head-group) -> core.  Core c handles batch c//4 and heads
3*(c%4) .. 3*(c%4)+2.  Q/K/V projections are computed per-core on the head
slice of the weights; the output projection is computed as a partial product
over the core's 192 combined-head dims and the 4 partials per batch are summed
on the host (the "all-reduce").

Device-side design (v1, ACT-paced):
  - The softmax exp stream on the Scalar (ACT) engine is the throughput
    floor (~427us busy: 384 x [128,1024] exp @ ~1.1us).  Everything else is
    scheduled around keeping ACT back-to-back:
      * flat (qc, h, pair) loop with cross-boundary scores prefetch so ACT
        never waits at head/chunk boundaries,
      * all projection / epilogue PE work is emitted through a fine-grained
        "filler pump" between the scores prefetch and the attn matmuls of
        each pair, so PE bubbles while waiting for exp are filled with
        useful work,
      * inputs stream on three parallel DMA queues (sync=xq, vector=xk,
        gpsimd=xv) with k/v chunk DMAs racing ahead of their projections.
  - Layouts are inherited from the baseline: host pre-transposes inputs to
    X^T [768, S] in bf16; q^T/k^T produced in [dk, S] with both 64-partition
    halves so K=64 scores matmuls row-pack two-at-a-time via tile_position;
    scores are computed transposed s^T[kpos, q]; v carries a ones-column so
    attn^T row 64 accumulates the softmax denominator; 1/8 scale folded into
    Wq; bq/bk folded into projection copies; bv/bo applied on the host.
  - attn^T accumulators are double-buffered in PSUM; the projection /
    output-projection / epilogue-transpose scratch shares one 2-buffer PSUM
    ring (epilogue tiles view it as bf16 via bitcast) so everything fits in
    the 8 PSUM banks: 2x scores(2) + 2x acc(1) + 2x misc(1).
  - attn output aT is drained to SBUF in bf16 (cheaper transposes + DVE).
"""

import os
import sys
from collections import deque

import numpy as np

for _p in ("/opt/trn_rl_repo", "/root/.axon_site/_ro/trn_rl_repo"):
    if _p not in sys.path and os.path.isdir(_p):
        sys.path.append(_p)

import concourse.bass as bass
import concourse.mybir as mybir
import concourse.tile as tile
from concourse.bass_utils import run_bass_kernel_spmd
from concourse.masks import make_identity

try:
    from ml_dtypes import bfloat16 as _bf16np
except ImportError:  # pragma: no cover
    _bf16np = np.dtype("bfloat16").type

F32 = mybir.dt.float32
BF16 = mybir.dt.bfloat16

D_MODEL = 768
N_HEADS_CORE = 3  # heads per core
DH = 192  # N_HEADS_CORE * 64
KCH = D_MODEL // 128  # contraction chunks for projections


def split_multi_waits(nc, max_waits=1):
    """This container's walrus rejects >1 semaphore wait per instruction
    (setupSyncWait).  Move excess waits onto same-engine NoOps just before
    the offending instruction."""
    n = 0
    for f in nc.m.functions:
        for bb in f.blocks:
            out = []
            for inst in bb.instructions:
                si = inst.sync_info
                if si is not None and si.on_wait and len(si.on_wait) > max_waits:
                    waits = list(si.on_wait)
                    for j, w in enumerate(waits[:-max_waits]):
                        out.append(
                            mybir.InstNoOp(
                                name=f"{inst.name}-wsplit{j}",
                                engine=inst.engine,
                                ins=[],
                                outs=[],
                                sync_info=mybir.SyncInfo(on_wait=[w], on_update=[]),
                            )
                        )
                    si.on_wait = waits[-max_waits:]
                    n += 1
                out.append(inst)
            bb.instructions = out
    return n


def build_nc(S, has_bq=True, has_bk=True, split=True):
    assert S % 512 == 0
    NQ = S // 512  # query chunks / projection chunks
    NT = S // 128  # kpos tiles
    NP2 = NT // 2  # kpos tile pairs for the h2 phase
    nc = bass.Bass()

    # chunk-contiguous layouts: one 6KB descriptor per partition per chunk
    xqa = nc.declare_dram_parameter("xqa", [S // 512, 128, KCH, 512], BF16, isOutput=False)
    xka = nc.declare_dram_parameter("xka", [S // 512, 128, KCH, 512], BF16, isOutput=False)
    xva = nc.declare_dram_parameter("xva", [S // 512, 128, KCH, 512], BF16, isOutput=False)
    wqa = nc.declare_dram_parameter("wqa", [128, KCH, DH], BF16, isOutput=False)
    wka = nc.declare_dram_parameter("wka", [128, KCH, DH], BF16, isOutput=False)
    wva = nc.declare_dram_parameter("wva", [128, KCH, DH], BF16, isOutput=False)
    wo0 = nc.declare_dram_parameter("wo0", [128, D_MODEL], BF16, isOutput=False)
    wo1 = nc.declare_dram_parameter("wo1", [64, D_MODEL], BF16, isOutput=False)
    bq = nc.declare_dram_parameter("bq", [DH, 1], F32, isOutput=False) if has_bq else None
    bk = nc.declare_dram_parameter("bk", [DH, 1], F32, isOutput=False) if has_bk else None
    part = nc.declare_dram_parameter("part", [S, D_MODEL], F32, isOutput=True)

    with tile.TileContext(nc) as tc:
        with (
            tc.tile_pool(name="consts", bufs=1) as consts,
            tc.tile_pool(name="persist", bufs=1) as persist,
            tc.tile_pool(name="xin", bufs=1) as xin,
            tc.tile_pool(name="probs", bufs=6) as probs_pool,
            tc.tile_pool(name="epi_sb", bufs=2) as epi_sb,
            tc.tile_pool(name="small", bufs=8) as small,
        ):
            # ---- constants (ride the idle-at-start ACT queue) ----
            id_bf16 = consts.tile([128, 128], BF16, tag="id_bf16")
            make_identity(nc, id_bf16)
            wq_sb = consts.tile([128, KCH, DH], BF16, tag="wq_sb")
            nc.scalar.dma_start(out=wq_sb, in_=wqa[:, :, :])
            wk_sb = consts.tile([128, KCH, DH], BF16, tag="wk_sb")
            nc.scalar.dma_start(out=wk_sb, in_=wka[:, :, :])
            wv_sb = consts.tile([128, KCH, DH], BF16, tag="wv_sb")
            nc.scalar.dma_start(out=wv_sb, in_=wva[:, :, :])
            bq_lo = bq_hi = bk_lo = bk_hi = None
            if has_bq:
                bq_lo = consts.tile([128, 1], F32, tag="bq_lo")
                nc.scalar.dma_start(out=bq_lo, in_=bq[0:128, :])
                bq_hi = consts.tile([64, 1], F32, tag="bq_hi")
                nc.scalar.dma_start(out=bq_hi, in_=bq[128:DH, :])
            if has_bk:
                bk_lo = consts.tile([128, 1], F32, tag="bk_lo")
                nc.scalar.dma_start(out=bk_lo, in_=bk[0:128, :])
                bk_hi = consts.tile([64, 1], F32, tag="bk_hi")
                nc.scalar.dma_start(out=bk_hi, in_=bk[128:DH, :])
            wo0_sb = consts.tile([128, D_MODEL], BF16, tag="wo0")
            wo1_sb = consts.tile([64, D_MODEL], BF16, tag="wo1")

            # ---- persistent activations, per 512-col chunk ----
            qTr = [[persist.tile([128, 512], BF16, tag=f"qTr{h}_{c}", name=f"qTr{h}_{c}")
                    for c in range(NQ)] for h in range(3)]
            kTr = [[persist.tile([128, 512], BF16, tag=f"kTr{h}_{c}", name=f"kTr{h}_{c}")
                    for c in range(NQ)] for h in range(3)]
            v_c = [persist.tile([128, 4, 3, 65], BF16, tag=f"v_{c}", name=f"v_{c}")
                   for c in range(NQ)]
            for c in range(NQ):
                nc.vector.memset(v_c[c][:, :, :, 64:65], 1.0)

            scratch = consts.tile([128, 8], BF16, tag="scratch")

            # per-chunk x views: (tile, column offset within tile)
            xq_t = [None] * NQ
            xk_t = [None] * NQ
            xv_t = [None] * NQ

            def dma_xq(c):
                t = xin.tile([128, KCH, 512], BF16, tag="xq", bufs=2, name=f"xq{c}")
                nc.sync.dma_start(out=t, in_=xqa[c])
                xq_t[c] = (t, 0)

            def dma_xk(c, eng=None):
                t = xin.tile([128, KCH, 512], BF16, tag="xk1", bufs=4, name=f"xk{c}")
                (eng or nc.gpsimd).dma_start(out=t, in_=xka[c])
                xk_t[c] = (t, 0)

            def dma_xv(c, eng=None):
                t = xin.tile([128, KCH, 512], BF16, tag="xv1", bufs=4, name=f"xv{c}")
                (eng or nc.sync).dma_start(out=t, in_=xva[c])
                xv_t[c] = (t, 0)

            with (
                tc.tile_pool(name="ps_big", bufs=2, space="PSUM") as ps_big,
                tc.tile_pool(name="ps_acc", bufs=2, space="PSUM") as ps_acc,
                tc.tile_pool(name="ps_misc", bufs=1, space="PSUM") as ps_misc,
                tc.tile_pool(name="ps_epi", bufs=1, space="PSUM") as ps_epi,
            ):
                epi_all = ps_epi.tile([128, 1024], BF16, tag="epi", name="epi_all")

                # ---------- projection helpers ----------
                def qk_halfproj_units(w_sb, xv, dst, blo, bhi, c, half):
                    """One 256-col half of a q/k projection chunk; 2-matmul
                    units.  Head rows: dst[0]=rows 0:64 primary lo,
                    dst[1]=rows 64:128 primary hi, dst[2]=rows 0:64 primary
                    lo + hi dup (only head 2 needs a duplicate)."""
                    x_t, xoff = xv
                    hc = bass.ds(half * 256, 256)
                    xc = bass.ds(xoff + half * 256, 256)
                    pst = ps_misc.tile([128, 512], F32, tag="misc", name="pst")
                    ps0 = pst[:, 0:256]
                    ps1 = pst[0:64, 256:512]
                    for k0 in range(0, KCH, 2):
                        for kk in (k0, k0 + 1):
                            nc.tensor.matmul(ps0, w_sb[:, kk, 0:128], x_t[:, kk, xc],
                                             start=(kk == 0), stop=(kk == KCH - 1),
                                             skip_group_check=True)
                        yield
                    for k0 in range(0, KCH, 2):
                        for kk in (k0, k0 + 1):
                            nc.tensor.matmul(ps1, w_sb[:, kk, 128:DH], x_t[:, kk, xc],
                                             start=(kk == 0), stop=(kk == KCH - 1),
                                             skip_group_check=True)
                        yield
                    if blo is not None:
                        nc.vector.tensor_scalar_add(dst[0][c][0:64, hc], ps0[0:64, :], blo[0:64])
                        nc.vector.tensor_scalar_add(dst[1][c][64:128, hc], ps0[64:128, :], blo[64:128])
                        nc.vector.tensor_scalar_add(dst[2][c][0:64, hc], ps1[0:64, :], bhi[0:64])
                    else:
                        nc.vector.tensor_copy(dst[0][c][0:64, hc], ps0[0:64, :])
                        nc.vector.tensor_copy(dst[1][c][64:128, hc], ps0[64:128, :])
                        nc.vector.tensor_copy(dst[2][c][0:64, hc], ps1[0:64, :])
                    nc.sync.dma_start(out=dst[2][c][64:128, hc], in_=dst[2][c][0:64, hc])
                    yield

                kproj_done = [False] * NQ
                vproj_done = [False] * NQ
                qproj_done = [False] * NQ

                def kproj_units(c):
                    yield from qk_halfproj_units(wk_sb, xk_t[c], kTr, bk_lo, bk_hi, c, 0)
                    yield from qk_halfproj_units(wk_sb, xk_t[c], kTr, bk_lo, bk_hi, c, 1)
                    kproj_done[c] = True

                def qproj_units(c):
                    yield from qk_halfproj_units(wq_sb, xq_t[c], qTr, bq_lo, bq_hi, c, 0)
                    yield from qk_halfproj_units(wq_sb, xq_t[c], qTr, bq_lo, bq_hi, c, 1)
                    qproj_done[c] = True

                def vproj_units(c):
                    x_t, xoff = xv_t[c]
                    for sub in range(4):
                        vps = ps_misc.tile([128, 512], F32, tag="misc", name="vps")
                        for kk in range(KCH):
                            nc.tensor.matmul(
                                vps[:, 0:DH],
                                x_t[:, kk, bass.ds(xoff + sub * 128, 128)],
                                wv_sb[:, kk, :],
                                start=(kk == 0), stop=(kk == KCH - 1),
                                skip_group_check=True,
                            )
                        nc.vector.tensor_copy(
                            v_c[c][:, sub, :, 0:64],
                            vps[:, 0:DH].rearrange("p (h d) -> p h d", h=3),
                        )
                        yield
                    vproj_done[c] = True

                # ---------- epilogue (split into head + fin stages) ----------
                # epi_all bank layout:
                #   head slots: (j%2)*512 + 66*h .. +65  (transposed attn + den)
                #   fin slots:  [256:512] (even j) / [768:1024] (odd j)
                comb_t = {}

                def epi_head_units(qc, h, aT_h):
                    # transpose + normalize must stay together per j: j and
                    # j+2 share a psum slot, so the reads (recip/mul) have to
                    # be emitted before the j+2 transpose reuses it
                    for j in range(4):
                        base = (j % 2) * 512 + 66 * h
                        ep = epi_all[:, bass.ds(base, 65)]
                        nc.tensor.transpose(ep, aT_h[:, bass.ts(j, 128)],
                                            id_bf16[0:65, 0:65])
                        if h == 0:
                            comb_t[(qc, j)] = epi_sb.tile(
                                [128, DH], BF16, tag="comb", bufs=8, name="comb")
                        comb = comb_t[(qc, j)]
                        rec = small.tile([128, 1], F32, tag="rec", name="rec")
                        nc.vector.reciprocal(rec, epi_all[:, base + 64: base + 65])
                        nc.vector.tensor_scalar_mul(
                            comb[:, bass.ts(h, 64)],
                            epi_all[:, bass.ds(base, 64)],
                            rec,
                        )
                        if j % 2 == 1:
                            yield
                    yield

                osb_t = {}

                def epi_fin_a_units(qc):
                    """Heads 0+1 share cT0, so their output-projection part
                    can run as soon as the h01 phase ends (pumped during the
                    h2 phase)."""
                    for j in range(4):
                        comb = comb_t[(qc, j)]
                        fb = 256 + (j % 2) * 512
                        fin0 = epi_all[:, bass.ds(fb, 128)]
                        nc.tensor.transpose(fin0, comb[:, 0:128], id_bf16)
                        yield
                        cT0 = epi_sb.tile([128, 128], BF16, tag="cT0", name="cT0")
                        nc.vector.tensor_copy(cT0, fin0)
                        o_sb = epi_sb.tile([128, D_MODEL], F32, tag="o_sb",
                                           bufs=4, name="o_sb")
                        osb_t[(qc, j)] = o_sb
                        op = ps_misc.tile([128, 512], F32, tag="misc", name="op")
                        nc.tensor.matmul(op[:, 0:512], cT0, wo0_sb[:, 0:512],
                                         start=True, stop=True, skip_group_check=True)
                        nc.vector.tensor_copy(o_sb[:, 0:512], op[:, 0:512])
                        yield
                        op2 = ps_misc.tile([128, 512], F32, tag="misc", name="op2")
                        nc.tensor.matmul(op2[:, 0:256], cT0, wo0_sb[:, 512:D_MODEL],
                                         start=True, stop=True, skip_group_check=True)
                        nc.vector.tensor_copy(o_sb[:, 512:D_MODEL], op2[:, 0:256])
                        yield

                def epi_fin_b_units(qc):
                    for j in range(4):
                        st = qc * 4 + j
                        comb = comb_t.pop((qc, j))
                        o_sb = osb_t.pop((qc, j))
                        fb = 256 + (j % 2) * 512
                        fin1 = epi_all[0:64, bass.ds(fb + 128, 128)]
                        nc.tensor.transpose(fin1, comb[:, 128:DH], id_bf16)
                        yield
                        cT1 = epi_sb.tile([64, 128], BF16, tag="cT1", name="cT1")
                        nc.vector.tensor_copy(cT1, fin1)
                        op = ps_misc.tile([128, 512], F32, tag="misc", name="opb")
                        nc.tensor.matmul(op[:, 0:512], cT1, wo1_sb[:, 0:512],
                                         start=True, stop=True, skip_group_check=True)
                        nc.vector.tensor_tensor(
                            out=o_sb[:, 0:512], in0=o_sb[:, 0:512],
                            in1=op[:, 0:512], op=mybir.AluOpType.add)
                        nc.sync.dma_start(
                            out=part[st * 128: (st + 1) * 128, 0:384],
                            in_=o_sb[:, 0:384],
                        )
                        yield
                        op2 = ps_misc.tile([128, 512], F32, tag="misc", name="opb2")
                        nc.tensor.matmul(op2[:, 0:256], cT1, wo1_sb[:, 512:D_MODEL],
                                         start=True, stop=True, skip_group_check=True)
                        nc.vector.tensor_tensor(
                            out=o_sb[:, 512:D_MODEL], in0=o_sb[:, 512:D_MODEL],
                            in1=op2[:, 0:256], op=mybir.AluOpType.add)
                        nc.gpsimd.dma_start(
                            out=part[st * 128: (st + 1) * 128, 384:D_MODEL],
                            in_=o_sb[:, 384:D_MODEL],
                        )
                        yield

                # ---------- scores ----------
                def scores_h01(t, qc):
                    """One kpos tile for heads 0+1, row-packed: h0 from the
                    primary lo halves, h1 from the primary hi halves - no
                    duplicate rows involved."""
                    sc = ps_big.tile([128, 1024], F32, tag="big", name="sc")
                    tc_ = bass.ts(t % 4, 128)
                    nc.tensor.matmul(
                        sc[:, 0:512], kTr[0][t // 4][0:64, tc_], qTr[0][qc][0:64, :],
                        start=True, stop=True, tile_position=(0, 0),
                    )
                    nc.tensor.matmul(
                        sc[:, 512:1024], kTr[1][t // 4][64:128, tc_], qTr[1][qc][64:128, :],
                        start=True, stop=True, tile_position=(64, 0),
                    )
                    return sc

                def scores_h2(t2, qc):
                    """Two kpos tiles for head 2, row-packed lo+hi (hi rows
                    are the duplicated copies)."""
                    t0, t1 = 2 * t2, 2 * t2 + 1
                    sc = ps_big.tile([128, 1024], F32, tag="big", name="sc")
                    nc.tensor.matmul(
                        sc[:, 0:512],
                        kTr[2][t0 // 4][0:64, bass.ts(t0 % 4, 128)],
                        qTr[2][qc][0:64, :],
                        start=True, stop=True, tile_position=(0, 0),
                    )
                    nc.tensor.matmul(
                        sc[:, 512:1024],
                        kTr[2][t1 // 4][64:128, bass.ts(t1 % 4, 128)],
                        qTr[2][qc][64:128, :],
                        start=True, stop=True, tile_position=(64, 0),
                    )
                    return sc

                # ---------- filler pump ----------
                fq = deque()

                def pump(n):
                    done = 0
                    while done < n and fq:
                        try:
                            next(fq[0])
                            done += 1
                        except StopIteration:
                            fq.popleft()

                def drain_until(flags, c):
                    while not flags[c] and fq:
                        try:
                            next(fq[0])
                        except StopIteration:
                            fq.popleft()
                    assert flags[c], f"filler queue drained but chunk {c} not emitted"

                # ---------- startup ----------
                # k/v stream round-robins across all three DMA-capable
                # queues (~75GB/s each); deadline order k0,v0,k1,v1,...
                dma_xq(0)
                queues = [nc.gpsimd, nc.sync, nc.scalar]
                qi = 0
                for c in range(NQ):
                    dma_xk(c, queues[qi % 3]); qi += 1
                    dma_xv(c, queues[qi % 3]); qi += 1
                for g in (qproj_units(0), kproj_units(0)):
                    for _ in g:
                        pass
                fq.append(vproj_units(0))
                fq.append(kproj_units(1))
                fq.append(vproj_units(1))
                for c in range(2, NQ):
                    fq.append(kproj_units(c))
                    fq.append(vproj_units(c))

                # ---------- flat ACT-paced main loop ----------
                # per qc: phase 'h01' = 32 single-tile pairs (heads 0+1),
                #         phase 'h2'  = 16 two-tile pairs (head 2)
                seq = []
                for qc in range(NQ):
                    seq.append((qc, "h01"))
                    seq.append((qc, "h2"))

                aT = [None, None, None]
                sc_cur = None

                def emit_scores(qc, phase, i):
                    if phase == "h01":
                        drain_until(kproj_done, i // 4)
                        drain_until(qproj_done, qc)
                        return scores_h01(i, qc)
                    drain_until(kproj_done, (2 * i) // 4)
                    drain_until(kproj_done, (2 * i + 1) // 4)
                    drain_until(qproj_done, qc)
                    return scores_h2(i, qc)

                sc_cur = emit_scores(0, "h01", 0)
                for si, (qc, phase) in enumerate(seq):
                    npair = NT if phase == "h01" else NP2
                    if phase == "h01":
                        acc0 = ps_acc.tile([65, 512], F32, tag="acc", name="acc0")
                        acc1 = ps_acc.tile([65, 512], F32, tag="acc", name="acc1")
                        if qc == 0:
                            pump_n = 6
                        else:
                            pump_n = 1
                        if qc >= 1:
                            fq.append(epi_fin_b_units(qc - 1))
                            if qc + 1 < NQ:
                                dma_xq(qc + 1)
                                fq.append(qproj_units(qc + 1))
                    else:
                        acc2 = ps_acc.tile([65, 512], F32, tag="acc", name="acc2")
                        pump_n = 3 if qc == 0 else 1
                        if qc == 0:
                            dma_xq(1)
                            fq.append(qproj_units(1))
                            nc.sync.dma_start(out=wo0_sb, in_=wo0[:, :])
                            nc.sync.dma_start(out=wo1_sb, in_=wo1[:, :])
                    for i in range(npair):
                        pr = probs_pool.tile([128, 1024], BF16, tag="pr")
                        nc.scalar.activation(
                            out=pr, in_=sc_cur,
                            func=mybir.ActivationFunctionType.Exp,
                        )
                        if i + 1 < npair:
                            sc_cur = emit_scores(qc, phase, i + 1)
                        elif si + 1 < len(seq):
                            nqc, nph = seq[si + 1]
                            sc_cur = emit_scores(nqc, nph, 0)
                        pump(pump_n)
                        if phase == "h01":
                            drain_until(vproj_done, i // 4)
                            nc.tensor.matmul(
                                acc0, v_c[i // 4][:, i % 4, 0, :], pr[:, 0:512],
                                start=(i == 0), stop=(i == npair - 1),
                                skip_group_check=True,
                            )
                            nc.tensor.matmul(
                                acc1, v_c[i // 4][:, i % 4, 1, :], pr[:, 512:1024],
                                start=(i == 0), stop=(i == npair - 1),
                                skip_group_check=True,
                            )
                        else:
                            t0, t1 = 2 * i, 2 * i + 1
                            drain_until(vproj_done, t0 // 4)
                            drain_until(vproj_done, t1 // 4)
                            nc.tensor.matmul(
                                acc2, v_c[t0 // 4][:, t0 % 4, 2, :], pr[:, 0:512],
                                start=(i == 0), stop=False, skip_group_check=True,
                            )
                            nc.tensor.matmul(
                                acc2, v_c[t1 // 4][:, t1 % 4, 2, :], pr[:, 512:1024],
                                start=False, stop=(i == npair - 1),
                                skip_group_check=True,
                            )
                    if phase == "h01":
                        aT[0] = epi_sb.tile([65, 512], BF16, tag="aT0", name="aT0")
                        aT[1] = epi_sb.tile([65, 512], BF16, tag="aT1", name="aT1")
                        nc.vector.tensor_copy(aT[0], acc0)
                        nc.vector.tensor_copy(aT[1], acc1)
                        fq.append(epi_head_units(qc, 0, aT[0]))
                        fq.append(epi_head_units(qc, 1, aT[1]))
                        fq.append(epi_fin_a_units(qc))
                    else:
                        aT[2] = epi_sb.tile([65, 512], BF16, tag="aT2", name="aT2")
                        nc.vector.tensor_copy(aT[2], acc2)
                        fq.append(epi_head_units(qc, 2, aT[2]))

                # drain remaining fillers, then the final chunk's fin stage
                fq.append(epi_fin_b_units(NQ - 1))
                pump(10**9)

    if split:
        split_multi_waits(nc)
    return nc


_NC_CACHE = {}


def _get_nc(S, has_bq, has_bk):
    key = (S, has_bq, has_bk)
    if key not in _NC_CACHE:
        _NC_CACHE[key] = build_nc(S, has_bq, has_bk)
    return _NC_CACHE[key]


def _arrange_x(X, S):
    """[S, D] input -> [S//512, 128, KCH, 512] chunk-contiguous bf16 layout
    (one contiguous 6KB run per (chunk, partition) for single-descriptor-
    per-partition DMAs).  arr[c, p, cc, s] = X.T[cc*128+p, c*512+s]."""
    xt = X.T.astype(_bf16np)  # [D, S]
    return np.ascontiguousarray(
        xt.reshape(KCH, 128, S // 512, 512).transpose(2, 1, 0, 3)
    )


def _arrange_w(Wslice):
    """[DH, D] weight slice -> [128, KCH, DH] bf16: w[p, cc, n] =
    W.T[cc*128+p, n]."""
    wt = Wslice.T.astype(_bf16np)  # [D, DH]
    return np.ascontiguousarray(wt.reshape(KCH, 128, DH).transpose(1, 0, 2))


def shard_inputs(Q, K, V, Wq, bq, Wk, bk, Wv, bv, Wo, bo, S):
    """Build the 8 per-core input maps (numpy, host-side shard+cast)."""
    has_bq = bool(np.any(bq))
    has_bk = bool(np.any(bk))
    in_maps = []
    xq_by_batch = [_arrange_x(Q[b], S) for b in range(Q.shape[0])]
    xk_by_batch = [_arrange_x(K[b], S) for b in range(Q.shape[0])]
    xv_by_batch = [_arrange_x(V[b], S) for b in range(Q.shape[0])]
    for c in range(8):
        b = c // 4
        r0 = 3 * (c % 4) * 64
        rows = slice(r0, r0 + DH)
        m = {
            "xqa": xq_by_batch[b],
            "xka": xk_by_batch[b],
            "xva": xv_by_batch[b],
            "wqa": _arrange_w(Wq[rows] / 8.0),
            "wka": _arrange_w(Wk[rows]),
            "wva": _arrange_w(Wv[rows]),
            "wo0": np.ascontiguousarray(Wo[:, rows][:, 0:128].T).astype(_bf16np),
            "wo1": np.ascontiguousarray(Wo[:, rows][:, 128:DH].T).astype(_bf16np),
        }
        if has_bq:
            m["bq"] = (bq[rows] / 8.0).reshape(DH, 1).astype(np.float32)
        if has_bk:
            m["bk"] = bk[rows].reshape(DH, 1).astype(np.float32)
        in_maps.append(m)
    return in_maps


def gather_output(results, Q, bv, Wo, bo):
    B, S = Q.shape[0], Q.shape[1]
    out = np.zeros((B, S, D_MODEL), np.float32)
    for c, r in enumerate(results):
        out[c // 4] += r["part"]
    out += (bv.astype(np.float32) @ Wo.T.astype(np.float32) + bo.astype(np.float32))[
        None, None, :
    ]
    return out


def kernel(Q, K, V, Wq, bq, Wk, bk, Wv, bv, Wo, bo, **run_kwargs):
    Q, K, V, Wq, bq, Wk, bk, Wv, bv, Wo, bo = (
        np.asarray(a) for a in (Q, K, V, Wq, bq, Wk, bk, Wv, bv, Wo, bo)
    )
    S = Q.shape[1]
    nc = _get_nc(S, bool(np.any(bq)), bool(np.any(bk)))
    in_maps = shard_inputs(Q, K, V, Wq, bq, Wk, bk, Wv, bv, Wo, bo, S)
    res = run_bass_kernel_spmd(nc, in_maps, core_ids=list(range(8)), **run_kwargs)
    out = gather_output(res.results, Q, bv, Wo, bo)
    kernel.last_results = res
    return out

